# revision 63
# baseline (speedup 1.0000x reference)
"""Trainium2 Bass kernel for nn_BaconAdditionReasoner (histogram_binning).

Math (per batch row):
    P1 = soft_perm(W1), P2 = soft_perm(W2)           (host, 10x10)
    l1 = p1 @ P1.T, l2 = p2 @ P2.T
    u = log(1-l1), v = log(1-l2)
    logprod[k] = sum_{i+j=k} max(u_i, v_j)
              = sum_{i+j=k} u_i + sum_{i+j=k} relu(v_j - u_i)
    e = exp(logprod);  out_k = (e_k - 1) / (sum_k e_k - 19)

Device dataflow (data parallel over 8 cores, 32768 rows/core):
  Front is feature-major: 4 bands of 20 feature rows at 32-aligned
  partitions (PE tile_position needs 32-aligned moving bases), batch on the
  free dim, in half-supertiles of 512 cols (2048 rows); processed in PAIRS
  (1024 cols) so Ln / Exp / the normalization tail amortize their
  per-instruction SBUF/PSUM access cost.
  - L matmul: blockdiag [117->116] f16 matmul per half; an extra constant
    input row adds 1e-6 to l, fusing the reference's upper clip so Ln's
    argument stays positive and sum(e) can never hit exactly 20
  - Ln (ACT): uv = log(1 - l) -> f16, one instr per pair
  - D matmuls: per band, [20->110] f16 pair-diff expansion in PSUM
    (v_j - u_i pairs + -u passthrough rows), two N=512 matmuls per band
    (ISA caps moving elements at 512)
  - relu: alternating ACT / DVE full-tile ops (GPSIMD cannot read PSUM
    and cannot run TensorScalarPtr on real HW, so the Pool engine is idle)
  - A-flip matmuls: per 128-col block, kt [110, 128] is the STATIONARY
    operand (LDWEIGHTS is free) and a [110, 20] +-1 matrix streams as the
    moving operand -> 20 cycles/block; batch-major logprod lands in two
    per-half [128, 320] PSUM tiles (a single 640-col f32 tile would make
    block 25 straddle the 2KB PSUM bank boundary, which corrupts on HW)
  - Exp (ACT) -> f32; 20-group reduce / recip / (e-1)*r on DVE per pair
  - output dumped partition-major [128, 5120] f16; host de-interleaves and
    drops the per-block pad column.

All HBM I/O and matmul moving operands are f16 (validated on the real
input distribution: max rel err ~2.7e-3 vs the 2e-2 gate).
"""

import numpy as np

# ---------------------------------------------------------------- constants
B = 262144
NCORES = 8
BC = B // NCORES            # 32768 rows per core
F = 512                     # batch columns per half-supertile (per band)
NB = 4                      # bands (32-aligned partition offsets)
ROWS_H = F * NB             # 2048 rows per half
NH = BC // ROWS_H           # 16 halves per core
NP = NH // 2                # 8 pairs
NCOLS = NH * F              # 8192 columns in pc
KB = 20                     # A-flip block width (19 k's + ISA pad col)
KCH = KB * (ROWS_H // 128)  # 320 output cols per half
KC = 2 * KCH                # 640 per pair
OCOLS = KC * NP             # 4864 output cols

# wk (constants, f16 [128, 256]) column layout
WL0, WL1 = 0, 116           # L blockdiag lhsT [116, 116]
WD0, WD1 = 116, 226         # D pair lhsT [20, 110] replicated per band
WA0, WA1 = 226, 246         # A-flip moving [110, 20] (col 19 zero pad)
WKC = 256                   # padded so DMA elem = 512 B

# relu engine split per band tile (4 tiles per pair, each [110, 2F] covering
# both halves), as (engine, c0, c1) over [0, 2F). GPSIMD cannot read PSUM on
# real HW, so only ACT ("A") and DVE ("D") can relu; the Pool engine instead
# takes the SBUF-only normalization ops.
RELU_SCHEME = [
    [("A", 0, 2 * F)],
    [("D", 0, 2 * F)],
    [("A", 0, 2 * F)],
    [("D", 0, 2 * F)],
]


def _soft_perm_np(W: np.ndarray) -> np.ndarray:
    W = W.astype(np.float32)
    lo = W.min(axis=1, keepdims=True)
    hi = W.max(axis=1, keepdims=True)
    Wn = (W - lo) / (hi - lo + np.float32(1e-8))
    return Wn / (Wn.sum(axis=1, keepdims=True) + np.float32(1e-8))


def _build_wk(P1n: np.ndarray, P2n: np.ndarray) -> np.ndarray:
    wk = np.zeros((128, WKC), dtype=np.float32)
    # --- L: lhsT[32q+d, 32q+e] = PP[e, d], PP = blockdiag(P1n, P2n).
    # Scale each row so its f16-rounded sum is <= 1-5e-6: with p <= 1.0 this
    # bounds l < 1-1e-6, so Ln(1-1e-6-l) (the fused reference clamp) never
    # sees a non-positive argument.
    def _scale_f16(P):
        P = P.astype(np.float64)
        for _ in range(4):
            s = np.float16(P.astype(np.float16)).astype(np.float64).sum(1)
            f = np.minimum(1.0, (1.0 - 5e-6) / np.maximum(s, 1e-30))
            P = P * f[:, None]
        return P.astype(np.float32)

    P1s, P2s = _scale_f16(P1n), _scale_f16(P2n)
    for q in range(NB):
        r = 32 * q
        wk[r : r + 10, r : r + 10] = P1s.T
        wk[r + 10 : r + 20, r + 10 : r + 20] = P2s.T
    # clamp row: pc row 116 carries 2**-10, weight 1.024e-3 -> l += 1e-6,
    # fusing the reference's upper clip (s <= 1-1e-6) into the L matmul
    wk[116, 0:116] = 1.024e-3
    # --- D: [20, 110]: pair col 10i+j gets v_j - u_i; col 100+e gets -u_e
    d = np.zeros((20, 110), dtype=np.float32)
    for i in range(10):
        for j in range(10):
            d[i, 10 * i + j] = -1.0
            d[10 + j, 10 * i + j] = 1.0
    for e in range(10):
        d[e, 100 + e] = -1.0
    for q in range(NB):
        wk[32 * q : 32 * q + 20, WD0:WD1] = d
    # --- A-flip moving [110, 20]: pair rows +1 at k=i+j; passthrough rows
    #     (-u values) -1 for k in [e, e+9]; col 19 all-zero (ISA pad: f16
    #     moving needs an even element count; exp(0)=1 is absorbed by -20)
    a = np.zeros((110, 20), dtype=np.float32)
    for i in range(10):
        for j in range(10):
            a[10 * i + j, i + j] = 1.0
    for e in range(10):
        a[100 + e, e : e + 10] = -1.0
    wk[0:110, WA0:WA1] = a
    return wk.astype(np.float16)


def _build_pc(p1c: np.ndarray, p2c: np.ndarray) -> np.ndarray:
    """[BC,10]x2 -> pc [116, NCOLS] f16: row 32q+e = feature e (u: e<10,
    v: 10<=e<20) of band q; col F*g+f = batch row ROWS_H*g + F*q + f."""
    pc = np.zeros((117, NCOLS), dtype=np.float16)
    pc[116, :] = np.float16(2.0 ** -10)
    x1 = p1c.reshape(NH, NB, F, 10)     # [g, q, f, d]
    x2 = p2c.reshape(NH, NB, F, 10)
    for q in range(NB):
        pc[32 * q : 32 * q + 10, :] = (
            x1[:, q].transpose(2, 0, 1).reshape(10, NCOLS).astype(np.float16)
        )
        pc[32 * q + 10 : 32 * q + 20, :] = (
            x2[:, q].transpose(2, 0, 1).reshape(10, NCOLS).astype(np.float16)
        )
    return pc


def _unpack_yraw(yraw: np.ndarray) -> np.ndarray:
    """yraw [128, OCOLS] f16 -> y [BC, 19] f32.
    yraw[p, KC*t + KB*(16h+4q+b) + k] = y[4096t+2048h+512q+128b+p, k]."""
    t = yraw.reshape(128, NP, 2, NB, 4, KB).transpose(1, 2, 3, 4, 0, 5)
    return np.ascontiguousarray(
        t.reshape(BC, KB)[:, 0:19].astype(np.float32)
    )


def _patch_act_tables():
    """Force Ln/Exp/Relu to resolve to the single set containing all three
    (natural_log_exp_and_others) so the activation table is loaded once."""
    import concourse.bacc as bacc
    from concourse import mybir

    if getattr(bacc, "_act_tables_patched", False):
        return
    orig = bacc.get_activation_tables
    AF = mybir.ActivationFunctionType
    shared = {AF.Ln, AF.Exp, AF.Relu}

    def patched(arch):
        tabs = orig(arch)
        if "natural_log_exp_and_others" in tabs:
            for name, funcs in tabs.items():
                if name != "natural_log_exp_and_others":
                    tabs[name] = set(funcs) - shared
        return tabs

    bacc.get_activation_tables = patched
    bacc._act_tables_patched = True


def build_bass():
    import concourse.bass as bass
    import concourse.bacc as bacc
    import concourse.tile as tile
    from concourse import mybir

    _patch_act_tables()
    f32 = mybir.dt.float32
    f16 = mybir.dt.float16
    AF = mybir.ActivationFunctionType
    ALU = mybir.AluOpType

    nc = bacc.Bacc("TRN2", target_bir_lowering=False)

    pc_d = nc.dram_tensor("pc", [117, NCOLS], f16, kind="ExternalInput")
    wk_d = nc.dram_tensor("wk", [128, WKC], f16, kind="ExternalInput")
    y_d = nc.dram_tensor("yraw", [128, OCOLS], f16, kind="ExternalOutput")

    with tile.TileContext(nc) as tc:
        with (
            tc.tile_pool(name="singles", bufs=1) as singles,
            tc.tile_pool(name="uv", bufs=2) as uv_p,
            tc.tile_pool(name="kt", bufs=4) as kt_p,
            tc.tile_pool(name="ee", bufs=2) as ee_p,
            tc.tile_pool(name="ss", bufs=2) as ss_p,
            tc.tile_pool(name="rr", bufs=2) as rr_p,
            tc.tile_pool(name="psL", bufs=1, space="PSUM") as psL,
            tc.tile_pool(name="psD", bufs=2, space="PSUM") as psD,
            tc.tile_pool(name="psA", bufs=1, space="PSUM") as psA,
        ):
            oo = singles.tile([128, OCOLS], f16)

            # prefetch the whole input up front (16 KB/partition); first chunk
            # is one pair so compute starts as early as possible
            pcc = singles.tile([117, NCOLS], f16)
            nc.sync.dma_start(pcc[:, 0:512], pc_d[:, 0:512])
            wk = singles.tile([128, WKC], f16)
            nc.sync.dma_start(wk[:, :], wk_d[:, :])
            for c0, c1 in [(512, 1024), (1024, 2048), (2048, 4096), (4096, 8192)]:
                nc.sync.dma_start(pcc[:, c0:c1], pc_d[:, c0:c1])

            # while the first DMAs land: preload the Ln/Exp/Relu activation
            # table and run dummy matmuls so the PE p-state ramps toward full
            # clock before real work arrives
            wz = singles.tile([20, F], f16)
            nc.vector.memset(wz[:, :], 0.0)
            wz2 = singles.tile([128, 8], f16)
            nc.scalar.activation(wz2[0:1, 0:1], wz[0:1, 0:1], AF.Ln)
            # NOTE: PE p-state warmup matmuls into pool-shared PSUM tiles
            # race with the real D matmuls on hardware (WAW ordering is not
            # enforced for the dead warmup writes), so no warmup is done.

            def emit_relu(scheme, dp):
                kt = kt_p.tile([110, 2 * F], f16)
                for eng, a0, a1 in scheme:
                    if eng == "A":
                        nc.scalar.activation(kt[:, a0:a1], dp[:, a0:a1], AF.Relu)
                    else:
                        nc.vector.tensor_scalar(
                            kt[:, a0:a1], dp[:, a0:a1], 0.0, None, op0=ALU.max
                        )
                return kt

            def emit_tail(pt):
                # normalization tail over oo cols [o0, o0+w); dma [d0, d1)
                e32, o0, w, dmarange = pt
                nb = w // KB
                ev = e32[:, 0:w].rearrange("p (b k) -> p b k", b=nb, k=KB)
                s32 = ss_p.tile([128, nb], f32)
                hh = nb // 2
                nc.vector.tensor_reduce(
                    s32[:, 0:hh], ev[:, 0:hh], axis=mybir.AxisListType.X,
                    op=ALU.add,
                )
                nc.vector.tensor_reduce(
                    s32[:, hh:nb], ev[:, hh:nb], axis=mybir.AxisListType.X,
                    op=ALU.add,
                )
                sm = ss_p.tile([128, nb], f32)
                nc.vector.tensor_scalar(
                    sm[:, :], s32[:, :], -20.0, None, op0=ALU.add
                )
                r32 = rr_p.tile([128, nb], f32)
                nc.vector.reciprocal(r32[:, :], sm[:, :])
                # out = (e - 1) * r  ==  (1-e)/(19-sum(e)), f16
                ov = oo[:, o0 : o0 + w].rearrange("p (b k) -> p b k", b=nb, k=KB)
                rb = r32[:, :].unsqueeze(-1).broadcast_to([128, nb, KB])
                nc.vector.scalar_tensor_tensor(
                    ov, ev, 1.0, rb, op0=ALU.subtract, op1=ALU.mult
                )
                if dmarange is not None:
                    d0, d1 = dmarange
                    nc.sync.dma_start(y_d[:, d0:d1], oo[:, d0:d1])

            def emit_front(t):
                # l = blockdiag(P1n, P2n) @ p ; one Ln per pair
                base = 2 * F * t
                lp = psL.tile([116, 2 * F], f32)
                for h in range(2):
                    nc.tensor.matmul(
                        lp[:, F * h : F * (h + 1)], wk[0:117, WL0:WL1],
                        pcc[0:117, base + F * h : base + F * (h + 1)],
                        start=True, stop=True,
                    )
                uvt = uv_p.tile([116, 2 * F], f16)
                if t == 0:
                    for h in range(2):
                        nc.scalar.activation(
                            uvt[:, F * h : F * (h + 1)],
                            lp[:, F * h : F * (h + 1)],
                            AF.Ln, bias=1.0, scale=-1.0,
                        )
                else:
                    nc.scalar.activation(
                        uvt[:, :], lp[:, :], AF.Ln, bias=1.0, scale=-1.0
                    )
                return uvt

            pending = None
            uvt_next = emit_front(0)
            for t in range(NP):
                uvt = uvt_next
                last = t == NP - 1
                # per-half ap tiles: a [128, 640] f32 tile would make
                # A-flip block 25 straddle the 2KB PSUM bank boundary, which
                # corrupts the matmul output on hardware
                ap_h = [
                    psA.tile([128, KCH], f32, name="aph") for _ in range(2)
                ]

                def emit_d(q):
                    # pair-diff grid for band q; two N=512 matmuls (the ISA
                    # caps a matmul's moving element count at 512)
                    r = 32 * q
                    dp = psD.tile([110, 2 * F], f32)
                    for h in range(2):
                        nc.tensor.matmul(
                            dp[:, F * h : F * (h + 1)],
                            wk[r : r + 20, WD0:WD1],
                            uvt[r : r + 20, F * h : F * (h + 1)],
                            start=True, stop=True, tile_position=(r, 0),
                        )
                    return dp

                def emit_aflip(q, kt):
                    # batch-major logprod: kt block stationary, [110,20] moving
                    for b in range(2 * F // 128):
                        h, bb = b // 4, b % 4
                        blk = 4 * q + bb
                        nc.tensor.matmul(
                            ap_h[h][:, KB * blk : KB * blk + KB],
                            kt[0:110, 128 * b : 128 * b + 128],
                            wk[0:110, WA0:WA1],
                            start=True, stop=True,
                        )

                # PE order: D0 D1 [A0] D2 [A1] D3 [A2] [A3] (psD bufs=2)
                dp0 = emit_d(0)
                kt0 = emit_relu(RELU_SCHEME[0], dp0)
                dp1 = emit_d(1)
                kt1 = emit_relu(RELU_SCHEME[1], dp1)
                emit_aflip(0, kt0)
                dp2 = emit_d(2)
                kt2 = emit_relu(RELU_SCHEME[2], dp2)
                emit_aflip(1, kt1)
                dp3 = emit_d(3)
                kt3 = emit_relu(RELU_SCHEME[3], dp3)
                emit_aflip(2, kt2)
                emit_aflip(3, kt3)

                if last:
                    # final pair: per-half (then per-quarter) Exp + immediate
                    # tails + split DMAs to shorten the closing serial chain
                    e32h = ee_p.tile([128, KCH], f32)
                    nc.scalar.activation(e32h[:, :], ap_h[0][:, :], AF.Exp)
                    if pending is not None:
                        emit_tail(pending)
                        pending = None
                    o0 = KC * t
                    emit_tail((e32h, o0, KCH, (o0, o0 + KCH)))
                    e32g = ee_p.tile([128, KCH], f32)
                    nc.scalar.activation(e32g[:, :], ap_h[1][:, :], AF.Exp)
                    o0 = KC * t + KCH
                    emit_tail((e32g, o0, KCH, (o0, o0 + KCH)))
                else:
                    # hoist next pair's front so ACT starts Ln(t+1) while it
                    # waits for this pair's last A-flips
                    uvt_next = emit_front(t + 1)
                    # e = exp(logprod) (f32: e-1 cancellation needs mantissa)
                    e32 = ee_p.tile([128, KC], f32)
                    for h in range(2):
                        nc.scalar.activation(
                            e32[:, KCH * h : KCH * (h + 1)], ap_h[h][:, :],
                            AF.Exp,
                        )
                    # defer this pair's tail until after the NEXT pair's grid
                    if pending is not None:
                        emit_tail(pending)
                    pending = (e32, KC * t, KC, (KC * t, KC * (t + 1)))
    nc.compile()
    return nc


_NC_CACHE = None


def kernel(p1, p2, W1, W2):
    global _NC_CACHE
    from concourse.bass_utils import run_bass_kernel_spmd

    P1n = _soft_perm_np(np.asarray(W1))
    P2n = _soft_perm_np(np.asarray(W2))
    wk = _build_wk(P1n, P2n)
    p1 = np.ascontiguousarray(np.asarray(p1, dtype=np.float32))
    p2 = np.ascontiguousarray(np.asarray(p2, dtype=np.float32))

    in_maps = []
    for c in range(NCORES):
        sl = slice(c * BC, (c + 1) * BC)
        in_maps.append({"pc": _build_pc(p1[sl], p2[sl]), "wk": wk})

    if _NC_CACHE is None:
        _NC_CACHE = build_bass()
    res = run_bass_kernel_spmd(_NC_CACHE, in_maps, core_ids=list(range(NCORES)))
    out = np.concatenate(
        [_unpack_yraw(res.results[c]["yraw"]) for c in range(NCORES)], axis=0
    )
    return out
